# revision 15
# baseline (speedup 1.0000x reference)
"""Fused LayerNorm + multi-head attention + output projection on 8 TRN2 NeuronCores.

Sharding: 2-way data parallel over batch x 4-way tensor parallel over heads.
Core c handles batch (c // 4), heads [4*(c%4) .. 4*(c%4)+4).

Device dataflow (everything transposed: host supplies x^T so the feature/
contraction dim always lands on SBUF partitions):
  - LayerNorm is folded into the QKV-projection epilogue:
      qkv^T[n,i] = rstd_i * (raw[n,i] - mu_i * wsum_n) (+ wb_n)
    with raw = W'^T x^T computed on raw x, row stats (mu, rstd) from
    PE ones-matmuls (which broadcast across partitions for free).
  - Scores are computed transposed (S^T[j,i]) so softmax'd probs feed the
    PV matmul without any transpose; two 64-dim heads are packed into the
    128 PE rows via tile_position row groups.
  - Softmax skips max-subtraction (values are bounded; a constant bias in
    the exp cancels in the normalization). The denominator comes from an
    extra ones-column appended to V (M=65 PV matmul).
  - The exp stream on the Scalar engine is the critical path. Emission is
    demand-driven: each attention iteration ensure()s exactly the QKV
    groups / V transposes it needs, and a drip scheduler pre-emits the
    rest into head-group 0's attention loop so the PE work overlaps the
    ACT-bound exp stream instead of serializing in a prologue.
  - Output projection produces partial^T per core; host sums the 4 TP
    partials per batch, adds b_out, and transposes back.
"""

import os
import sys
from collections import deque

import numpy as np

for _p in ("/root/.axon_site", "/root/.axon_site/_ro/trn_rl_repo",
           "/root/.axon_site/_ro/pypackages", "/opt/trn_rl_repo"):
    if os.path.isdir(_p) and _p not in sys.path:
        sys.path.append(_p)

B = 2
N = 2048
DIM = 1024
HEADS = 16
DIM_HEAD = 64
INNER = HEADS * DIM_HEAD
HEADS_PER_CORE = 4          # 4-way tensor parallel on heads
N_CORES = 8
EPS = 1e-5
EXP_BIAS = -4.0             # constant subtracted inside exp; cancels in softmax

KT = DIM // 128             # 8 k-tiles of the contraction dim
IC = 4                      # i-chunks of 512 over N=2048
ICW = N // IC               # 512
JT = N // 128               # 16 j-tiles
NQKV = 3 * HEADS_PER_CORE * DIM_HEAD   # 768 local qkv columns
NT = NQKV // 128            # 6 n-tiles: [q01, q23, k01, k23, v01, v23]
MT = DIM // 128             # 8 output m-tiles

_COMPILED = {}


def _build(has_wb):
    import concourse.bass as bass
    import concourse.mybir as mybir
    from concourse import bacc, tile
    from concourse.masks import make_identity
    from contextlib import ExitStack

    f32 = mybir.dt.float32
    bf16 = mybir.dt.bfloat16
    AF = mybir.ActivationFunctionType
    ALU = mybir.AluOpType

    nc = bacc.Bacc("TRN2", target_bir_lowering=False, debug=False,
                   num_devices=N_CORES)

    xT_d = nc.dram_tensor("xT", [DIM, N], bf16, kind="ExternalInput")
    wqkv_d = nc.dram_tensor("wqkv", [DIM, NQKV], bf16, kind="ExternalInput")
    wout_d = nc.dram_tensor("wout", [HEADS_PER_CORE * DIM_HEAD, DIM], bf16,
                            kind="ExternalInput")
    wsum_d = nc.dram_tensor("wsum", [NQKV, 1], f32, kind="ExternalInput")
    wb_d = nc.dram_tensor("wb", [NQKV, 1], f32, kind="ExternalInput")
    out_d = nc.dram_tensor("outT", [DIM, N], f32, kind="ExternalOutput")

    with ExitStack() as ctx:
        tc = ctx.enter_context(tile.TileContext(nc))
        cst = ctx.enter_context(tc.tile_pool(name="cst", bufs=1))
        xp = ctx.enter_context(tc.tile_pool(name="xp", bufs=KT))
        wp = ctx.enter_context(tc.tile_pool(name="wp", bufs=KT))
        qkp = ctx.enter_context(tc.tile_pool(name="qk", bufs=1))
        vtp = ctx.enter_context(tc.tile_pool(name="vt", bufs=1))
        vaugp = ctx.enter_context(tc.tile_pool(name="vaug", bufs=JT))
        bcp = ctx.enter_context(tc.tile_pool(name="bc", bufs=1))
        scp = ctx.enter_context(tc.tile_pool(name="sc", bufs=2))
        ep = ctx.enter_context(tc.tile_pool(name="ep", bufs=4))
        onp = ctx.enter_context(tc.tile_pool(name="on", bufs=2 * IC))
        otp = ctx.enter_context(tc.tile_pool(name="ot", bufs=3))
        smp = ctx.enter_context(tc.tile_pool(name="sm", bufs=2))
        # single PSUM pool, 8 banks total:
        #   tag "s": 2 x [128,1024] (2 banks each) -> 4 banks (score tiles)
        #   tag "o": 3 x [128,512]                 -> 3 banks
        #            (PV accumulators + out-projection tiles)
        #   tag "g": 1 x [128,512]                 -> 1 bank
        #            (LN stats, qkv groups, v transposes)
        psum = ctx.enter_context(tc.tile_pool(name="psum", bufs=2,
                                              space="PSUM"))

        # ---- constants & weight loads ----
        ones = cst.tile([128, 128], bf16)
        nc.vector.memset(ones[:], 1.0)
        eps_t = cst.tile([128, 1], f32, tag="eps")
        nc.vector.memset(eps_t[:], EPS)
        ebias_t = cst.tile([128, 1], f32, tag="ebias")
        nc.vector.memset(ebias_t[:], EXP_BIAS)
        ident = cst.tile([128, 128], bf16)
        make_identity(nc, ident[:])
        wsum_t = cst.tile([128, NT], f32)
        wb_t = cst.tile([128, NT], f32)
        for nt in range(NT):
            nc.sync.dma_start(wsum_t[:, nt:nt + 1],
                              wsum_d[nt * 128:(nt + 1) * 128, :])
            if has_wb:
                nc.sync.dma_start(wb_t[:, nt:nt + 1],
                                  wb_d[nt * 128:(nt + 1) * 128, :])

        # x loads split per i-chunk so the first stats matmuls can start
        # as soon as the first 128KB chunks land.
        xt = []
        for k in range(KT):
            t = xp.tile([128, N], bf16, tag="xt", name=f"xt{k}")
            for c in range(IC):
                nc.sync.dma_start(
                    t[:, c * ICW:(c + 1) * ICW],
                    xT_d[k * 128:(k + 1) * 128, c * ICW:(c + 1) * ICW])
            xt.append(t)
        wt = []
        for k in range(KT):
            t = wp.tile([128, NQKV], bf16, tag="wt", name=f"wt{k}")
            nc.sync.dma_start(t[:], wqkv_d[k * 128:(k + 1) * 128, :])
            wt.append(t)
        wo = []
        for d in range(2):
            t = cst.tile([128, DIM], bf16, tag=f"wo{d}", name=f"wo{d}")
            nc.sync.dma_start(t[:], wout_d[d * 128:(d + 1) * 128, :])
            wo.append(t)

        # ---- persistent activation tiles ----
        mu_bc = bcp.tile([128, N], f32, tag="mu")
        nrstd_bc = bcp.tile([128, N], f32, tag="nrstd")
        q01 = qkp.tile([128, N], bf16, tag="q01")
        q23 = qkp.tile([128, N], bf16, tag="q23")
        k01 = qkp.tile([128, N], bf16, tag="k01")
        k23 = qkp.tile([128, N], bf16, tag="k23")
        vT = [vtp.tile([128, N], bf16, tag=f"vt{i}", name=f"vt{i}")
              for i in range(2)]
        qkv_dst = [q01, q23, k01, k23, vT[0], vT[1]]
        vaug = [[None] * JT for _ in range(2)]

        # ---- demand-driven emission ----
        emitted = set()

        def ensure(key, fn, *args):
            if key not in emitted:
                emitted.add(key)
                fn(*args)

        def stats_emit(ic):
            """LN row stats for one i-chunk: mu_bc, nrstd_bc columns."""
            isl = slice(ic * ICW, (ic + 1) * ICW)
            sum_ps = psum.tile([128, ICW], f32, tag="g", name="sum_ps")
            for k in range(KT):
                nc.tensor.matmul(sum_ps[:], ones[:], xt[k][:, isl],
                                 start=(k == 0), stop=(k == KT - 1))
            nc.vector.tensor_scalar_mul(mu_bc[:, isl], sum_ps[:], 1.0 / DIM)
            sq_ps = psum.tile([128, ICW], f32, tag="g", name="sq_ps")
            for k in range(KT):
                x2 = scp.tile([128, ICW], bf16, tag="x2", bufs=3, name="x2")
                nc.vector.tensor_mul(x2[:], xt[k][:, isl], xt[k][:, isl])
                nc.tensor.matmul(sq_ps[:], ones[:], x2[:],
                                 start=(k == 0), stop=(k == KT - 1))
            msq = scp.tile([128, ICW], f32, tag="msq", bufs=1, name="msq")
            nc.vector.tensor_scalar_mul(msq[:], sq_ps[:], 1.0 / DIM)
            mu2 = scp.tile([128, ICW], f32, tag="mu2", bufs=1, name="mu2")
            nc.vector.tensor_mul(mu2[:], mu_bc[:, isl], mu_bc[:, isl])
            var = scp.tile([128, ICW], f32, tag="var", bufs=1, name="var")
            nc.vector.tensor_sub(var[:], msq[:], mu2[:])
            std = scp.tile([128, ICW], f32, tag="std", bufs=1, name="std")
            nc.scalar.activation(std[:], var[:], AF.Sqrt, bias=eps_t[:, 0:1])
            rstd = scp.tile([128, ICW], f32, tag="rstd", bufs=1, name="rstd")
            rsc = scp.tile([128, ICW], f32, tag="rsc", bufs=1, name="rsc")
            nc.vector.reciprocal_approx_accurate(rstd[:], std[:], rsc[:])
            nc.vector.tensor_scalar_mul(nrstd_bc[:, isl], rstd[:], -1.0)

        def qkv_emit(nt, ic):
            ensure(("st", ic), stats_emit, ic)
            nsl = slice(nt * 128, (nt + 1) * 128)
            isl = slice(ic * ICW, (ic + 1) * ICW)
            ps = psum.tile([128, ICW], f32, tag="g", name="qkv_ps")
            for k in range(KT):
                nc.tensor.matmul(ps[:], wt[k][:, nsl], xt[k][:, isl],
                                 start=(k == 0), stop=(k == KT - 1))
            # (mu*wsum - raw) * (-rstd) [+ wb]
            tmp = scp.tile([128, ICW], f32, tag="fix", bufs=3, name="fix")
            nc.vector.scalar_tensor_tensor(
                tmp[:], mu_bc[:, isl], wsum_t[:, nt:nt + 1], ps[:],
                op0=ALU.mult, op1=ALU.subtract)
            dst = qkv_dst[nt][:, isl]
            nc.vector.tensor_mul(dst, tmp[:], nrstd_bc[:, isl])
            if has_wb:
                nc.vector.tensor_scalar_add(dst, dst, wb_t[:, nt:nt + 1])

        def tp_emit(d, j):
            """v^T -> v_aug[d][j]: [v_h | 1] blocks for the two local heads."""
            ensure(("g", 4 + d, j // 4), qkv_emit, 4 + d, j // 4)
            va = vaugp.tile([128, 2 * 65], bf16, tag=f"vaug{d}",
                            name=f"vaug{d}_{j}", bufs=JT)
            vaug[d][j] = va
            nc.vector.memset(va[:, 64:2 * 65:65], 1.0)
            tp = psum.tile([128, 128], bf16, tag="g", name="tp")
            nc.tensor.transpose(tp[:], vT[d][:, j * 128:(j + 1) * 128],
                                ident[:])
            nc.vector.tensor_copy(va[:, 0:64], tp[:, 0:64])
            nc.vector.tensor_copy(va[:, 65:129], tp[:, 64:128])

        def ensure_qkv(nt, ic):
            ensure(("g", nt, ic), qkv_emit, nt, ic)

        def ensure_tp(d, j):
            ensure(("tp", d, j), tp_emit, d, j)

        # drip schedule: pre-emit upcoming work (deadline order) so it
        # overlaps the exp stream instead of bursting at ensure points.
        drip = deque()
        for c in range(1, IC):
            drip.append(("g", 2, c))
            for j in range(4 * c, 4 * c + 4):
                drip.append(("tp", 0, j))
        for c in range(1, IC):
            drip.append(("g", 0, c))
        for c in range(IC):
            drip.append(("g", 3, c))
            for j in range(4 * c, 4 * c + 4):
                drip.append(("tp", 1, j))
        for c in range(IC):
            drip.append(("g", 1, c))

        def drip_one():
            while drip:
                key = drip.popleft()
                if key in emitted:
                    continue
                if key[0] == "g":
                    ensure_qkv(key[1], key[2])
                else:
                    ensure_tp(key[1], key[2])
                return

        # ---- head: minimum work before the exp stream can start ----
        ensure_qkv(0, 0)          # q01 first chunk (pulls stats(0))
        ensure_qkv(2, 0)          # k01 first j-blocks
        for j in range(4):
            ensure_tp(0, j)       # v01 first j-blocks

        # ---- attention + output projection ----
        qt_pair = [q01, q23]
        kt_pair = [k01, k23]
        o_norm = [[onp.tile([128, ICW], bf16, tag="onorm",
                            name=f"onorm{p}_{i}") for i in range(IC)]
                  for p in range(2)]
        for pair in range(2):
            qt = qt_pair[pair]
            kt = kt_pair[pair]
            for ic in range(IC):
                isl = slice(ic * ICW, (ic + 1) * ICW)
                ensure_qkv(pair, ic)
                o_ps = [psum.tile([128, ICW], f32, tag="o", name="o_ps")
                        for _ in range(2)]
                for j in range(JT):
                    jsl = slice(j * 128, (j + 1) * 128)
                    ensure_qkv(2 + pair, j // 4)
                    ensure_tp(pair, j)
                    s_ps = psum.tile([128, 2 * ICW], f32, tag="s", name="s_ps")
                    e_t = ep.tile([128, 2 * ICW], bf16, tag="e", name="e_t")
                    for hh in range(2):
                        psl = slice(hh * 64, (hh + 1) * 64)
                        nc.tensor.matmul(s_ps[:, hh * ICW:(hh + 1) * ICW],
                                         kt[psl, jsl], qt[psl, isl])
                    nc.scalar.activation(e_t[:], s_ps[:], AF.Exp,
                                         bias=ebias_t[:, 0:1])
                    for hh in range(2):
                        nc.tensor.matmul(
                            o_ps[hh][0:65, :],
                            vaug[pair][j][:, 65 * hh:65 * hh + 65],
                            e_t[:, hh * ICW:(hh + 1) * ICW],
                            start=(j == 0), stop=(j == JT - 1))
                    drip_one()
                # normalize: O[d,i] / l_i  (l is row 64 of o_ps)
                for hh in range(2):
                    o_sb = smp.tile([65, ICW], f32, tag="osb", name="osb")
                    nc.vector.tensor_copy(o_sb[:], o_ps[hh][0:65, :])
                    lrow = smp.tile([1, ICW], f32, tag="lrow", name="lrow")
                    nc.sync.dma_start(lrow[:], o_sb[64:65, :])
                    linv = smp.tile([1, ICW], f32, tag="linv", name="linv")
                    lsc = smp.tile([1, ICW], f32, tag="lsc", name="lsc")
                    nc.vector.reciprocal_approx_accurate(linv[:], lrow[:],
                                                         lsc[:])
                    lbc = smp.tile([64, ICW], f32, tag="lbc", name="lbc")
                    nc.gpsimd.partition_broadcast(lbc[:], linv[:])
                    if hh == 0:
                        nc.vector.tensor_mul(o_norm[pair][ic][0:64, :],
                                             o_sb[0:64, :], lbc[:])
                    else:
                        ob = smp.tile([64, ICW], bf16, tag="ob", name="ob")
                        nc.vector.tensor_mul(ob[:], o_sb[0:64, :], lbc[:])
                        nc.sync.dma_start(o_norm[pair][ic][64:128, :], ob[:])
                # output projection for this i-chunk (after both pairs done)
                if pair == 1:
                    for mt in range(MT):
                        msl = slice(mt * 128, (mt + 1) * 128)
                        pps = psum.tile([128, ICW], f32, tag="o", name="pj_ps")
                        for d in range(2):
                            nc.tensor.matmul(pps[:], wo[d][:, msl],
                                             o_norm[d][ic][:],
                                             start=(d == 0), stop=(d == 1))
                        ot = otp.tile([128, ICW], f32, tag="ot", name="ot")
                        nc.vector.tensor_copy(ot[:], pps[:])
                        nc.sync.dma_start(out_d[msl, isl], ot[:])

    nc.compile()
    return nc


def _get_compiled(has_wb):
    key = bool(has_wb)
    if key not in _COMPILED:
        _COMPILED[key] = _build(key)
    return _COMPILED[key]


def _shard_inputs(x, ln_gamma, ln_beta, w_qkv, w_out):
    """Build per-core input maps (all host-side layout work, no math on x)."""
    import ml_dtypes
    bf = ml_dtypes.bfloat16

    x = np.ascontiguousarray(np.asarray(x, np.float32))
    g = np.asarray(ln_gamma, np.float32)
    be = np.asarray(ln_beta, np.float32)
    wq = np.asarray(w_qkv, np.float32)
    wo = np.asarray(w_out, np.float32)

    scale = DIM_HEAD ** (-0.5)
    wq_g = wq * g[:, None]            # gamma folded
    wq_g[:, :INNER] *= scale          # attention scale folded into W_q
    wb_full = be @ wq                 # beta contribution
    wb_full[:INNER] *= scale

    in_maps = []
    for c in range(N_CORES):
        b = c // HEADS_PER_CORE
        grp = c % HEADS_PER_CORE
        heads = [HEADS_PER_CORE * grp + t for t in range(HEADS_PER_CORE)]
        # column order: [q01, q23, k01, k23, v01, v23] pair-tiles
        cols = []
        for which in range(3):        # q, k, v
            for h in heads:
                lo = which * INNER + h * DIM_HEAD
                cols.append(np.arange(lo, lo + DIM_HEAD))
        cols = np.concatenate(cols)
        wqkv_c = np.ascontiguousarray(wq_g[:, cols])
        # bf16-round the weights before computing wsum so the LN-fold
        # correction matches what the device matmul actually sums.
        wqkv_bf = wqkv_c.astype(bf)
        wsum_c = wqkv_bf.astype(np.float32).sum(axis=0).reshape(NQKV, 1)
        wb_c = wb_full[cols].reshape(NQKV, 1)
        rows = np.concatenate([np.arange(h * DIM_HEAD, (h + 1) * DIM_HEAD)
                               for h in heads])
        wout_c = np.ascontiguousarray(wo[rows, :])
        in_maps.append({
            "xT": np.ascontiguousarray(x[b].T).astype(bf),
            "wqkv": wqkv_bf,
            "wout": wout_c.astype(bf),
            "wsum": np.ascontiguousarray(wsum_c),
            "wb": np.ascontiguousarray(wb_c),
        })
    return in_maps


def _run(inputs, trace=False):
    from concourse.bass_utils import run_bass_kernel_spmd

    in_maps = _shard_inputs(inputs["x"], inputs["ln_gamma"],
                            inputs["ln_beta"], inputs["w_qkv"],
                            inputs["w_out"])
    has_wb = bool(np.max(np.abs(in_maps[0]["wb"])) > 0)
    nc = _get_compiled(has_wb)
    res = run_bass_kernel_spmd(nc, in_maps, core_ids=list(range(N_CORES)),
                               trace=trace)
    b_out = np.asarray(inputs["b_out"], np.float32)
    outs = []
    for b in range(B):
        acc = np.zeros((DIM, N), np.float32)
        for grp in range(HEADS_PER_CORE):
            acc += res.results[b * HEADS_PER_CORE + grp]["outT"]
        outs.append(acc.T + b_out)
    out = np.stack(outs).astype(np.float32)
    return out, res


def kernel(**inputs):
    out, _ = _run(inputs, trace=False)
    return out


# revision 16
# speedup vs baseline: 1.1182x; 1.1182x over previous
"""Fused LayerNorm + multi-head attention + output projection on 8 TRN2 NeuronCores.

Sharding: 2-way data parallel over batch x 4-way tensor parallel over heads.
Core c handles batch (c // 4), heads [4*(c%4) .. 4*(c%4)+4).

Device dataflow (everything transposed: host supplies x^T so the feature/
contraction dim always lands on SBUF partitions):
  - LayerNorm is folded into the QKV-projection epilogue:
      qkv^T[n,i] = rstd_i * (raw[n,i] - mu_i * wsum_n) (+ wb_n)
    with raw = W'^T x^T computed on raw x, row stats (mu, rstd) from
    PE ones-matmuls (which broadcast across partitions for free).
  - Scores are computed transposed (S^T[j,i]) so softmax'd probs feed the
    PV matmul without any transpose; two 64-dim heads are packed into the
    128 PE rows via tile_position row groups.
  - Softmax skips max-subtraction (values are bounded; a constant bias in
    the exp cancels in the normalization). The denominator comes from an
    extra ones-column appended to V (M=65 PV matmul).
  - The exp stream on the Scalar engine is the critical path. Emission is
    demand-driven: each attention iteration ensure()s exactly the QKV
    groups / V transposes it needs, and a drip scheduler pre-emits the
    rest into head-group 0's attention loop so the PE work overlaps the
    ACT-bound exp stream instead of serializing in a prologue.
  - Output projection produces partial^T per core; host sums the 4 TP
    partials per batch, adds b_out, and transposes back.
"""

import os
import sys
from collections import deque

import numpy as np

for _p in ("/root/.axon_site", "/root/.axon_site/_ro/trn_rl_repo",
           "/root/.axon_site/_ro/pypackages", "/opt/trn_rl_repo"):
    if os.path.isdir(_p) and _p not in sys.path:
        sys.path.append(_p)

B = 2
N = 2048
DIM = 1024
HEADS = 16
DIM_HEAD = 64
INNER = HEADS * DIM_HEAD
HEADS_PER_CORE = 4          # 4-way tensor parallel on heads
N_CORES = 8
EPS = 1e-5
EXP_BIAS = -4.0             # constant subtracted inside exp; cancels in softmax

KT = DIM // 128             # 8 k-tiles of the contraction dim
IC = 4                      # i-chunks of 512 over N=2048
ICW = N // IC               # 512
JT = N // 128               # 16 j-tiles
NQKV = 3 * HEADS_PER_CORE * DIM_HEAD   # 768 local qkv columns
NT = NQKV // 128            # 6 n-tiles: [q01, q23, k01, k23, v01, v23]
MT = DIM // 128             # 8 output m-tiles

_COMPILED = {}


def _build(has_wb):
    import concourse.bass as bass
    import concourse.mybir as mybir
    from concourse import bacc, tile
    from concourse.masks import make_identity
    from contextlib import ExitStack

    f32 = mybir.dt.float32
    bf16 = mybir.dt.bfloat16
    AF = mybir.ActivationFunctionType
    ALU = mybir.AluOpType

    nc = bacc.Bacc("TRN2", target_bir_lowering=False, debug=False,
                   num_devices=N_CORES)

    xT_d = nc.dram_tensor("xT", [DIM, N], bf16, kind="ExternalInput")
    wqkv_d = nc.dram_tensor("wqkv", [DIM, NQKV], bf16, kind="ExternalInput")
    wout_d = nc.dram_tensor("wout", [HEADS_PER_CORE * DIM_HEAD, DIM], bf16,
                            kind="ExternalInput")
    wsum_d = nc.dram_tensor("wsum", [NQKV, 1], f32, kind="ExternalInput")
    wb_d = nc.dram_tensor("wb", [NQKV, 1], f32, kind="ExternalInput")
    out_d = nc.dram_tensor("outT", [DIM, N], f32, kind="ExternalOutput")

    with ExitStack() as ctx:
        tc = ctx.enter_context(tile.TileContext(nc))
        cst = ctx.enter_context(tc.tile_pool(name="cst", bufs=1))
        xp = ctx.enter_context(tc.tile_pool(name="xp", bufs=KT))
        wp = ctx.enter_context(tc.tile_pool(name="wp", bufs=KT))
        qkp = ctx.enter_context(tc.tile_pool(name="qk", bufs=1))
        vtp = ctx.enter_context(tc.tile_pool(name="vt", bufs=1))
        vaugp = ctx.enter_context(tc.tile_pool(name="vaug", bufs=JT))
        bcp = ctx.enter_context(tc.tile_pool(name="bc", bufs=1))
        scp = ctx.enter_context(tc.tile_pool(name="sc", bufs=2))
        ep = ctx.enter_context(tc.tile_pool(name="ep", bufs=4))
        onp = ctx.enter_context(tc.tile_pool(name="on", bufs=2 * IC))
        otp = ctx.enter_context(tc.tile_pool(name="ot", bufs=3))
        smp = ctx.enter_context(tc.tile_pool(name="sm", bufs=2))
        # single PSUM pool, 8 banks total:
        #   tag "s": 2 x [128,1024] (2 banks each) -> 4 banks (score tiles)
        #   tag "o": 2 x [128,512]                 -> 2 banks (PV accums)
        #   tag "g": 2 x [128,512]                 -> 2 banks
        #            (LN stats, qkv groups, v transposes, out projection)
        psum = ctx.enter_context(tc.tile_pool(name="psum", bufs=2,
                                              space="PSUM"))

        # ---- constants & weight loads ----
        ones = cst.tile([128, 128], bf16)
        nc.vector.memset(ones[:], 1.0)
        eps_t = cst.tile([128, 1], f32, tag="eps")
        nc.vector.memset(eps_t[:], EPS)
        ebias_t = cst.tile([128, 1], f32, tag="ebias")
        nc.vector.memset(ebias_t[:], EXP_BIAS)
        ident = cst.tile([128, 128], bf16)
        make_identity(nc, ident[:])
        wsum_t = cst.tile([128, NT], f32)
        wb_t = cst.tile([128, NT], f32)
        for nt in range(NT):
            nc.sync.dma_start(wsum_t[:, nt:nt + 1],
                              wsum_d[nt * 128:(nt + 1) * 128, :])
            if has_wb:
                nc.sync.dma_start(wb_t[:, nt:nt + 1],
                                  wb_d[nt * 128:(nt + 1) * 128, :])

        # x loads split per i-chunk so the first stats matmuls can start
        # as soon as the first 128KB chunks land.
        xt = []
        for k in range(KT):
            t = xp.tile([128, N], bf16, tag="xt", name=f"xt{k}")
            for c in range(IC):
                nc.sync.dma_start(
                    t[:, c * ICW:(c + 1) * ICW],
                    xT_d[k * 128:(k + 1) * 128, c * ICW:(c + 1) * ICW])
            xt.append(t)
        wt = []
        for k in range(KT):
            t = wp.tile([128, NQKV], bf16, tag="wt", name=f"wt{k}")
            nc.sync.dma_start(t[:], wqkv_d[k * 128:(k + 1) * 128, :])
            wt.append(t)
        wo = []
        for d in range(2):
            t = cst.tile([128, DIM], bf16, tag=f"wo{d}", name=f"wo{d}")
            nc.sync.dma_start(t[:], wout_d[d * 128:(d + 1) * 128, :])
            wo.append(t)

        # ---- persistent activation tiles ----
        mu_bc = bcp.tile([128, N], f32, tag="mu")
        nrstd_bc = bcp.tile([128, N], f32, tag="nrstd")
        q01 = qkp.tile([128, N], bf16, tag="q01")
        q23 = qkp.tile([128, N], bf16, tag="q23")
        k01 = qkp.tile([128, N], bf16, tag="k01")
        k23 = qkp.tile([128, N], bf16, tag="k23")
        vT = [vtp.tile([128, N], bf16, tag=f"vt{i}", name=f"vt{i}")
              for i in range(2)]
        qkv_dst = [q01, q23, k01, k23, vT[0], vT[1]]
        vaug = [[None] * JT for _ in range(2)]

        # ---- demand-driven emission ----
        emitted = set()

        def ensure(key, fn, *args):
            if key not in emitted:
                emitted.add(key)
                fn(*args)

        def stats_emit(ic):
            """LN row stats for one i-chunk: mu_bc, nrstd_bc columns."""
            isl = slice(ic * ICW, (ic + 1) * ICW)
            sum_ps = psum.tile([128, ICW], f32, tag="g", name="sum_ps")
            for k in range(KT):
                nc.tensor.matmul(sum_ps[:], ones[:], xt[k][:, isl],
                                 start=(k == 0), stop=(k == KT - 1))
            nc.vector.tensor_scalar_mul(mu_bc[:, isl], sum_ps[:], 1.0 / DIM)
            sq_ps = psum.tile([128, ICW], f32, tag="g", name="sq_ps")
            for k in range(KT):
                x2 = scp.tile([128, ICW], bf16, tag="x2", bufs=3, name="x2")
                nc.vector.tensor_mul(x2[:], xt[k][:, isl], xt[k][:, isl])
                nc.tensor.matmul(sq_ps[:], ones[:], x2[:],
                                 start=(k == 0), stop=(k == KT - 1))
            msq = scp.tile([128, ICW], f32, tag="msq", bufs=1, name="msq")
            nc.vector.tensor_scalar_mul(msq[:], sq_ps[:], 1.0 / DIM)
            mu2 = scp.tile([128, ICW], f32, tag="mu2", bufs=1, name="mu2")
            nc.vector.tensor_mul(mu2[:], mu_bc[:, isl], mu_bc[:, isl])
            var = scp.tile([128, ICW], f32, tag="var", bufs=1, name="var")
            nc.vector.tensor_sub(var[:], msq[:], mu2[:])
            std = scp.tile([128, ICW], f32, tag="std", bufs=1, name="std")
            nc.scalar.activation(std[:], var[:], AF.Sqrt, bias=eps_t[:, 0:1])
            rstd = scp.tile([128, ICW], f32, tag="rstd", bufs=1, name="rstd")
            rsc = scp.tile([128, ICW], f32, tag="rsc", bufs=1, name="rsc")
            nc.vector.reciprocal_approx_accurate(rstd[:], std[:], rsc[:])
            nc.vector.tensor_scalar_mul(nrstd_bc[:, isl], rstd[:], -1.0)

        def qkv_emit(nt, ic):
            ensure(("st", ic), stats_emit, ic)
            nsl = slice(nt * 128, (nt + 1) * 128)
            isl = slice(ic * ICW, (ic + 1) * ICW)
            ps = psum.tile([128, ICW], f32, tag="g", name="qkv_ps")
            for k in range(KT):
                nc.tensor.matmul(ps[:], wt[k][:, nsl], xt[k][:, isl],
                                 start=(k == 0), stop=(k == KT - 1))
            # (mu*wsum - raw) * (-rstd) [+ wb]
            tmp = scp.tile([128, ICW], f32, tag="fix", bufs=3, name="fix")
            nc.vector.scalar_tensor_tensor(
                tmp[:], mu_bc[:, isl], wsum_t[:, nt:nt + 1], ps[:],
                op0=ALU.mult, op1=ALU.subtract)
            dst = qkv_dst[nt][:, isl]
            nc.vector.tensor_mul(dst, tmp[:], nrstd_bc[:, isl])
            if has_wb:
                nc.vector.tensor_scalar_add(dst, dst, wb_t[:, nt:nt + 1])

        def tp_emit(d, j):
            """v^T -> v_aug[d][j]: [v_h | 1] blocks for the two local heads."""
            ensure(("g", 4 + d, j // 4), qkv_emit, 4 + d, j // 4)
            va = vaugp.tile([128, 2 * 65], bf16, tag=f"vaug{d}",
                            name=f"vaug{d}_{j}", bufs=JT)
            vaug[d][j] = va
            nc.vector.memset(va[:, 64:2 * 65:65], 1.0)
            tp = psum.tile([128, 128], bf16, tag="g", name="tp")
            nc.tensor.transpose(tp[:], vT[d][:, j * 128:(j + 1) * 128],
                                ident[:])
            nc.vector.tensor_copy(va[:, 0:64], tp[:, 0:64])
            nc.vector.tensor_copy(va[:, 65:129], tp[:, 64:128])

        def ensure_qkv(nt, ic):
            ensure(("g", nt, ic), qkv_emit, nt, ic)

        def ensure_tp(d, j):
            ensure(("tp", d, j), tp_emit, d, j)

        # drip schedule: pre-emit upcoming work (deadline order) so it
        # overlaps the exp stream instead of bursting at ensure points.
        drip = deque()
        for c in range(1, IC):
            drip.append(("g", 2, c))
            for j in range(4 * c, 4 * c + 4):
                drip.append(("tp", 0, j))
        for c in range(1, IC):
            drip.append(("g", 0, c))
        for c in range(IC):
            drip.append(("g", 3, c))
            for j in range(4 * c, 4 * c + 4):
                drip.append(("tp", 1, j))
        for c in range(IC):
            drip.append(("g", 1, c))

        def drip_one():
            while drip:
                key = drip.popleft()
                if key in emitted:
                    continue
                if key[0] == "g":
                    ensure_qkv(key[1], key[2])
                else:
                    ensure_tp(key[1], key[2])
                return

        # ---- head: minimum work before the exp stream can start ----
        ensure_qkv(0, 0)          # q01 first chunk (pulls stats(0))
        ensure_qkv(2, 0)          # k01 first j-blocks
        for j in range(4):
            ensure_tp(0, j)       # v01 first j-blocks
        ensure_qkv(2, 1)
        for j in range(4, 8):
            ensure_tp(0, j)

        # ---- attention + output projection ----
        qt_pair = [q01, q23]
        kt_pair = [k01, k23]
        o_norm = [[onp.tile([128, ICW], bf16, tag="onorm",
                            name=f"onorm{p}_{i}") for i in range(IC)]
                  for p in range(2)]
        for pair in range(2):
            qt = qt_pair[pair]
            kt = kt_pair[pair]
            for ic in range(IC):
                isl = slice(ic * ICW, (ic + 1) * ICW)
                ensure_qkv(pair, ic)
                o_ps = [psum.tile([128, ICW], f32, tag="o", name="o_ps")
                        for _ in range(2)]
                for j in range(JT):
                    jsl = slice(j * 128, (j + 1) * 128)
                    ensure_qkv(2 + pair, j // 4)
                    ensure_tp(pair, j)
                    s_ps = psum.tile([128, 2 * ICW], f32, tag="s", name="s_ps")
                    e_t = ep.tile([128, 2 * ICW], bf16, tag="e", name="e_t")
                    for hh in range(2):
                        psl = slice(hh * 64, (hh + 1) * 64)
                        nc.tensor.matmul(s_ps[:, hh * ICW:(hh + 1) * ICW],
                                         kt[psl, jsl], qt[psl, isl])
                    nc.scalar.activation(e_t[:], s_ps[:], AF.Exp,
                                         bias=ebias_t[:, 0:1])
                    for hh in range(2):
                        nc.tensor.matmul(
                            o_ps[hh][0:65, :],
                            vaug[pair][j][:, 65 * hh:65 * hh + 65],
                            e_t[:, hh * ICW:(hh + 1) * ICW],
                            start=(j == 0), stop=(j == JT - 1))
                    drip_one()
                # normalize: O[d,i] / l_i  (l is row 64 of o_ps)
                for hh in range(2):
                    o_sb = smp.tile([65, ICW], f32, tag="osb", name="osb")
                    nc.scalar.copy(o_sb[:], o_ps[hh][0:65, :])
                    lrow = smp.tile([1, ICW], f32, tag="lrow", name="lrow")
                    nc.sync.dma_start(lrow[:], o_sb[64:65, :])
                    linv = smp.tile([1, ICW], f32, tag="linv", name="linv")
                    lsc = smp.tile([1, ICW], f32, tag="lsc", name="lsc")
                    nc.vector.reciprocal_approx_accurate(linv[:], lrow[:],
                                                         lsc[:])
                    lbc = smp.tile([64, ICW], f32, tag="lbc", name="lbc")
                    nc.gpsimd.partition_broadcast(lbc[:], linv[:])
                    if hh == 0:
                        nc.vector.tensor_mul(o_norm[pair][ic][0:64, :],
                                             o_sb[0:64, :], lbc[:])
                    else:
                        ob = smp.tile([64, ICW], bf16, tag="ob", name="ob")
                        nc.vector.tensor_mul(ob[:], o_sb[0:64, :], lbc[:])
                        nc.sync.dma_start(o_norm[pair][ic][64:128, :], ob[:])
                # output projection for this i-chunk (after both pairs done)
                if pair == 1:
                    for mt in range(MT):
                        msl = slice(mt * 128, (mt + 1) * 128)
                        pps = psum.tile([128, ICW], f32, tag="g", name="pj_ps")
                        for d in range(2):
                            nc.tensor.matmul(pps[:], wo[d][:, msl],
                                             o_norm[d][ic][:],
                                             start=(d == 0), stop=(d == 1))
                        ot = otp.tile([128, ICW], f32, tag="ot", name="ot")
                        nc.vector.tensor_copy(ot[:], pps[:])
                        nc.sync.dma_start(out_d[msl, isl], ot[:])

    nc.compile()
    return nc


def _get_compiled(has_wb):
    key = bool(has_wb)
    if key not in _COMPILED:
        _COMPILED[key] = _build(key)
    return _COMPILED[key]


def _shard_inputs(x, ln_gamma, ln_beta, w_qkv, w_out):
    """Build per-core input maps (all host-side layout work, no math on x)."""
    import ml_dtypes
    bf = ml_dtypes.bfloat16

    x = np.ascontiguousarray(np.asarray(x, np.float32))
    g = np.asarray(ln_gamma, np.float32)
    be = np.asarray(ln_beta, np.float32)
    wq = np.asarray(w_qkv, np.float32)
    wo = np.asarray(w_out, np.float32)

    scale = DIM_HEAD ** (-0.5)
    wq_g = wq * g[:, None]            # gamma folded
    wq_g[:, :INNER] *= scale          # attention scale folded into W_q
    wb_full = be @ wq                 # beta contribution
    wb_full[:INNER] *= scale

    in_maps = []
    for c in range(N_CORES):
        b = c // HEADS_PER_CORE
        grp = c % HEADS_PER_CORE
        heads = [HEADS_PER_CORE * grp + t for t in range(HEADS_PER_CORE)]
        # column order: [q01, q23, k01, k23, v01, v23] pair-tiles
        cols = []
        for which in range(3):        # q, k, v
            for h in heads:
                lo = which * INNER + h * DIM_HEAD
                cols.append(np.arange(lo, lo + DIM_HEAD))
        cols = np.concatenate(cols)
        wqkv_c = np.ascontiguousarray(wq_g[:, cols])
        # bf16-round the weights before computing wsum so the LN-fold
        # correction matches what the device matmul actually sums.
        wqkv_bf = wqkv_c.astype(bf)
        wsum_c = wqkv_bf.astype(np.float32).sum(axis=0).reshape(NQKV, 1)
        wb_c = wb_full[cols].reshape(NQKV, 1)
        rows = np.concatenate([np.arange(h * DIM_HEAD, (h + 1) * DIM_HEAD)
                               for h in heads])
        wout_c = np.ascontiguousarray(wo[rows, :])
        in_maps.append({
            "xT": np.ascontiguousarray(x[b].T).astype(bf),
            "wqkv": wqkv_bf,
            "wout": wout_c.astype(bf),
            "wsum": np.ascontiguousarray(wsum_c),
            "wb": np.ascontiguousarray(wb_c),
        })
    return in_maps


def _run(inputs, trace=False):
    from concourse.bass_utils import run_bass_kernel_spmd

    in_maps = _shard_inputs(inputs["x"], inputs["ln_gamma"],
                            inputs["ln_beta"], inputs["w_qkv"],
                            inputs["w_out"])
    has_wb = bool(np.max(np.abs(in_maps[0]["wb"])) > 0)
    nc = _get_compiled(has_wb)
    res = run_bass_kernel_spmd(nc, in_maps, core_ids=list(range(N_CORES)),
                               trace=trace)
    b_out = np.asarray(inputs["b_out"], np.float32)
    outs = []
    for b in range(B):
        acc = np.zeros((DIM, N), np.float32)
        for grp in range(HEADS_PER_CORE):
            acc += res.results[b * HEADS_PER_CORE + grp]["outT"]
        outs.append(acc.T + b_out)
    out = np.stack(outs).astype(np.float32)
    return out, res


def kernel(**inputs):
    out, _ = _run(inputs, trace=False)
    return out


# revision 17
# speedup vs baseline: 1.1852x; 1.0598x over previous
"""Fused LayerNorm + multi-head attention + output projection on 8 TRN2 NeuronCores.

Sharding: 2-way data parallel over batch x 4-way tensor parallel over heads.
Core c handles batch (c // 4), heads [4*(c%4) .. 4*(c%4)+4).

Device dataflow (everything transposed: host supplies x^T so the feature/
contraction dim always lands on SBUF partitions):
  - LayerNorm is folded into the QKV-projection epilogue:
      qkv^T[n,i] = rstd_i * (raw[n,i] - mu_i * wsum_n) (+ wb_n)
    with raw = W'^T x^T computed on raw x, row stats (mu, rstd) from
    PE ones-matmuls (which broadcast across partitions for free).
  - Scores are computed transposed (S^T[j,i]) so softmax'd probs feed the
    PV matmul without any transpose; two 64-dim heads are packed into the
    128 PE rows via tile_position row groups.
  - Softmax skips max-subtraction (values are bounded; a constant bias in
    the exp cancels in the normalization). The denominator comes from an
    extra ones-column appended to V (M=65 PV matmul).
  - The exp stream on the Scalar engine is the critical path. Emission is
    demand-driven: each attention iteration ensure()s exactly the QKV
    groups / V transposes it needs, and a drip scheduler pre-emits the
    rest into head-group 0's attention loop so the PE work overlaps the
    ACT-bound exp stream instead of serializing in a prologue.
  - Output projection produces partial^T per core; host sums the 4 TP
    partials per batch, adds b_out, and transposes back.
"""

import os
import sys
from collections import deque

import numpy as np

for _p in ("/root/.axon_site", "/root/.axon_site/_ro/trn_rl_repo",
           "/root/.axon_site/_ro/pypackages", "/opt/trn_rl_repo"):
    if os.path.isdir(_p) and _p not in sys.path:
        sys.path.append(_p)

B = 2
N = 2048
DIM = 1024
HEADS = 16
DIM_HEAD = 64
INNER = HEADS * DIM_HEAD
HEADS_PER_CORE = 4          # 4-way tensor parallel on heads
N_CORES = 8
EPS = 1e-5
EXP_BIAS = -4.0             # constant subtracted inside exp; cancels in softmax

KT = DIM // 128             # 8 k-tiles of the contraction dim
IC = 4                      # i-chunks of 512 over N=2048
ICW = N // IC               # 512
JT = N // 128               # 16 j-tiles
NQKV = 3 * HEADS_PER_CORE * DIM_HEAD   # 768 local qkv columns
NT = NQKV // 128            # 6 n-tiles: [q01, q23, k01, k23, v01, v23]
MT = DIM // 128             # 8 output m-tiles

_COMPILED = {}


def _build(has_wb):
    import concourse.bass as bass
    import concourse.mybir as mybir
    from concourse import bacc, tile
    from concourse.masks import make_identity
    from contextlib import ExitStack

    f32 = mybir.dt.float32
    bf16 = mybir.dt.bfloat16
    AF = mybir.ActivationFunctionType
    ALU = mybir.AluOpType

    nc = bacc.Bacc("TRN2", target_bir_lowering=False, debug=False,
                   num_devices=N_CORES)

    xT_d = nc.dram_tensor("xT", [DIM, N], bf16, kind="ExternalInput")
    wqkv_d = nc.dram_tensor("wqkv", [DIM, NQKV], bf16, kind="ExternalInput")
    wout_d = nc.dram_tensor("wout", [HEADS_PER_CORE * DIM_HEAD, DIM], bf16,
                            kind="ExternalInput")
    wsum_d = nc.dram_tensor("wsum", [NQKV, 1], f32, kind="ExternalInput")
    wb_d = nc.dram_tensor("wb", [NQKV, 1], f32, kind="ExternalInput")
    out_d = nc.dram_tensor("outT", [DIM, N], bf16, kind="ExternalOutput")

    with ExitStack() as ctx:
        tc = ctx.enter_context(tile.TileContext(nc))
        cst = ctx.enter_context(tc.tile_pool(name="cst", bufs=1))
        xp = ctx.enter_context(tc.tile_pool(name="xp", bufs=KT))
        wp = ctx.enter_context(tc.tile_pool(name="wp", bufs=KT))
        qkp = ctx.enter_context(tc.tile_pool(name="qk", bufs=1))
        vtp = ctx.enter_context(tc.tile_pool(name="vt", bufs=1))
        vaugp = ctx.enter_context(tc.tile_pool(name="vaug", bufs=JT))
        bcp = ctx.enter_context(tc.tile_pool(name="bc", bufs=1))
        scp = ctx.enter_context(tc.tile_pool(name="sc", bufs=2))
        ep = ctx.enter_context(tc.tile_pool(name="ep", bufs=4))
        onp = ctx.enter_context(tc.tile_pool(name="on", bufs=2 * IC))
        otp = ctx.enter_context(tc.tile_pool(name="ot", bufs=3))
        smp = ctx.enter_context(tc.tile_pool(name="sm", bufs=2))
        # single PSUM pool, 8 banks total:
        #   tag "s": 2 x [128,1024] (2 banks each) -> 4 banks (score tiles)
        #   tag "o": 2 x [128,512]                 -> 2 banks (PV accums)
        #   tag "g": 2 x [128,512]                 -> 2 banks
        #            (LN stats, qkv groups, v transposes, out projection)
        psum = ctx.enter_context(tc.tile_pool(name="psum", bufs=2,
                                              space="PSUM"))

        # ---- constants & weight loads ----
        ones = cst.tile([128, 128], bf16)
        nc.vector.memset(ones[:], 1.0)
        eps_t = cst.tile([128, 1], f32, tag="eps")
        nc.vector.memset(eps_t[:], EPS)
        ebias_t = cst.tile([128, 1], f32, tag="ebias")
        nc.vector.memset(ebias_t[:], EXP_BIAS)
        ident = cst.tile([128, 128], bf16)
        make_identity(nc, ident[:])
        wsum_t = cst.tile([128, NT], f32)
        wb_t = cst.tile([128, NT], f32)
        for nt in range(NT):
            nc.sync.dma_start(wsum_t[:, nt:nt + 1],
                              wsum_d[nt * 128:(nt + 1) * 128, :])
            if has_wb:
                nc.sync.dma_start(wb_t[:, nt:nt + 1],
                                  wb_d[nt * 128:(nt + 1) * 128, :])

        # x loads split per i-chunk so the first stats matmuls can start
        # as soon as the first 128KB chunks land.
        xt = []
        for k in range(KT):
            t = xp.tile([128, N], bf16, tag="xt", name=f"xt{k}")
            for c in range(IC):
                nc.sync.dma_start(
                    t[:, c * ICW:(c + 1) * ICW],
                    xT_d[k * 128:(k + 1) * 128, c * ICW:(c + 1) * ICW])
            xt.append(t)
        wt = []
        for k in range(KT):
            t = wp.tile([128, NQKV], bf16, tag="wt", name=f"wt{k}")
            nc.sync.dma_start(t[:], wqkv_d[k * 128:(k + 1) * 128, :])
            wt.append(t)
        wo = []
        for d in range(2):
            t = cst.tile([128, DIM], bf16, tag=f"wo{d}", name=f"wo{d}")
            nc.sync.dma_start(t[:], wout_d[d * 128:(d + 1) * 128, :])
            wo.append(t)

        # ---- persistent activation tiles ----
        mu_bc = bcp.tile([128, N], f32, tag="mu")
        nrstd_bc = bcp.tile([128, N], f32, tag="nrstd")
        q01 = qkp.tile([128, N], bf16, tag="q01")
        q23 = qkp.tile([128, N], bf16, tag="q23")
        k01 = qkp.tile([128, N], bf16, tag="k01")
        k23 = qkp.tile([128, N], bf16, tag="k23")
        vT = [vtp.tile([128, N], bf16, tag=f"vt{i}", name=f"vt{i}")
              for i in range(2)]
        qkv_dst = [q01, q23, k01, k23, vT[0], vT[1]]
        vaug = [[None] * JT for _ in range(2)]

        # ---- demand-driven emission ----
        emitted = set()

        def ensure(key, fn, *args):
            if key not in emitted:
                emitted.add(key)
                fn(*args)

        def stats_emit(ic):
            """LN row stats for one i-chunk: mu_bc, nrstd_bc columns."""
            isl = slice(ic * ICW, (ic + 1) * ICW)
            sum_ps = psum.tile([128, ICW], f32, tag="g", name="sum_ps")
            for k in range(KT):
                nc.tensor.matmul(sum_ps[:], ones[:], xt[k][:, isl],
                                 start=(k == 0), stop=(k == KT - 1))
            nc.vector.tensor_scalar_mul(mu_bc[:, isl], sum_ps[:], 1.0 / DIM)
            sq_ps = psum.tile([128, ICW], f32, tag="g", name="sq_ps")
            for k in range(KT):
                x2 = scp.tile([128, ICW], bf16, tag="x2", bufs=3, name="x2")
                nc.vector.tensor_mul(x2[:], xt[k][:, isl], xt[k][:, isl])
                nc.tensor.matmul(sq_ps[:], ones[:], x2[:],
                                 start=(k == 0), stop=(k == KT - 1))
            msq = scp.tile([128, ICW], f32, tag="msq", bufs=1, name="msq")
            nc.vector.tensor_scalar_mul(msq[:], sq_ps[:], 1.0 / DIM)
            mu2 = scp.tile([128, ICW], f32, tag="mu2", bufs=1, name="mu2")
            nc.vector.tensor_mul(mu2[:], mu_bc[:, isl], mu_bc[:, isl])
            var = scp.tile([128, ICW], f32, tag="var", bufs=1, name="var")
            nc.vector.tensor_sub(var[:], msq[:], mu2[:])
            std = scp.tile([128, ICW], f32, tag="std", bufs=1, name="std")
            nc.scalar.activation(std[:], var[:], AF.Sqrt, bias=eps_t[:, 0:1])
            rstd = scp.tile([128, ICW], f32, tag="rstd", bufs=1, name="rstd")
            rsc = scp.tile([128, ICW], f32, tag="rsc", bufs=1, name="rsc")
            nc.vector.reciprocal_approx_accurate(rstd[:], std[:], rsc[:])
            nc.vector.tensor_scalar_mul(nrstd_bc[:, isl], rstd[:], -1.0)

        def qkv_emit(nt, ic):
            ensure(("st", ic), stats_emit, ic)
            nsl = slice(nt * 128, (nt + 1) * 128)
            isl = slice(ic * ICW, (ic + 1) * ICW)
            ps = psum.tile([128, ICW], f32, tag="g", name="qkv_ps")
            for k in range(KT):
                nc.tensor.matmul(ps[:], wt[k][:, nsl], xt[k][:, isl],
                                 start=(k == 0), stop=(k == KT - 1))
            # (mu*wsum - raw) * (-rstd) [+ wb]
            tmp = scp.tile([128, ICW], f32, tag="fix", bufs=3, name="fix")
            nc.vector.scalar_tensor_tensor(
                tmp[:], mu_bc[:, isl], wsum_t[:, nt:nt + 1], ps[:],
                op0=ALU.mult, op1=ALU.subtract)
            dst = qkv_dst[nt][:, isl]
            nc.vector.tensor_mul(dst, tmp[:], nrstd_bc[:, isl])
            if has_wb:
                nc.vector.tensor_scalar_add(dst, dst, wb_t[:, nt:nt + 1])

        def tp_emit(d, j):
            """v^T -> v_aug[d][j]: [v_h | 1] blocks for the two local heads."""
            ensure(("g", 4 + d, j // 4), qkv_emit, 4 + d, j // 4)
            va = vaugp.tile([128, 2 * 65], bf16, tag=f"vaug{d}",
                            name=f"vaug{d}_{j}", bufs=JT)
            vaug[d][j] = va
            nc.vector.memset(va[:, 64:2 * 65:65], 1.0)
            tp = psum.tile([128, 128], bf16, tag="g", name="tp")
            nc.tensor.transpose(tp[:], vT[d][:, j * 128:(j + 1) * 128],
                                ident[:])
            nc.vector.tensor_copy(va[:, 0:64], tp[:, 0:64])
            nc.vector.tensor_copy(va[:, 65:129], tp[:, 64:128])

        def ensure_qkv(nt, ic):
            ensure(("g", nt, ic), qkv_emit, nt, ic)

        def ensure_tp(d, j):
            ensure(("tp", d, j), tp_emit, d, j)

        # drip schedule: pre-emit upcoming work (deadline order) so it
        # overlaps the exp stream instead of bursting at ensure points.
        # Only modest pre-emission: each pair mostly streams its own k/v/
        # transpose groups via the JIT ensures in the attention loop.
        drip = deque()
        drip.append(("st", 1))
        drip.append(("g", 2, 2))
        for j in range(8, 12):
            drip.append(("tp", 0, j))
        drip.append(("st", 2))
        drip.append(("g", 2, 3))
        for j in range(12, 16):
            drip.append(("tp", 0, j))
        drip.append(("st", 3))
        drip.append(("g", 1, 0))
        drip.append(("g", 1, 1))
        drip.append(("g", 1, 2))
        drip.append(("g", 1, 3))

        def drip_one():
            while drip:
                key = drip.popleft()
                if key in emitted:
                    continue
                if key[0] == "g":
                    ensure_qkv(key[1], key[2])
                elif key[0] == "st":
                    ensure(key, stats_emit, key[1])
                else:
                    ensure_tp(key[1], key[2])
                return

        # ---- head: minimum work before the exp stream can start ----
        ensure_qkv(0, 0)          # q01 first chunk (pulls stats(0))
        ensure_qkv(2, 0)          # k01 first j-blocks
        for j in range(4):
            ensure_tp(0, j)       # v01 first j-blocks
        ensure_qkv(2, 1)
        for j in range(4, 8):
            ensure_tp(0, j)

        # ---- attention + output projection ----
        qt_pair = [q01, q23]
        kt_pair = [k01, k23]
        o_norm = [[onp.tile([128, ICW], bf16, tag="onorm",
                            name=f"onorm{p}_{i}") for i in range(IC)]
                  for p in range(2)]
        for pair in range(2):
            qt = qt_pair[pair]
            kt = kt_pair[pair]
            for ic in range(IC):
                isl = slice(ic * ICW, (ic + 1) * ICW)
                ensure_qkv(pair, ic)
                o_ps = [psum.tile([128, ICW], f32, tag="o", name="o_ps")
                        for _ in range(2)]
                for j in range(JT):
                    jsl = slice(j * 128, (j + 1) * 128)
                    ensure_qkv(2 + pair, j // 4)
                    ensure_tp(pair, j)
                    s_ps = psum.tile([128, 2 * ICW], f32, tag="s", name="s_ps")
                    e_t = ep.tile([128, 2 * ICW], bf16, tag="e", name="e_t")
                    for hh in range(2):
                        psl = slice(hh * 64, (hh + 1) * 64)
                        nc.tensor.matmul(s_ps[:, hh * ICW:(hh + 1) * ICW],
                                         kt[psl, jsl], qt[psl, isl])
                    nc.scalar.activation(e_t[:], s_ps[:], AF.Exp,
                                         bias=ebias_t[:, 0:1])
                    for hh in range(2):
                        nc.tensor.matmul(
                            o_ps[hh][0:65, :],
                            vaug[pair][j][:, 65 * hh:65 * hh + 65],
                            e_t[:, hh * ICW:(hh + 1) * ICW],
                            start=(j == 0), stop=(j == JT - 1))
                    if not (pair == 0 and ic == 0 and j % 2 == 0):
                        drip_one()
                # normalize: O[d,i] / l_i  (l is row 64 of o_ps)
                for hh in range(2):
                    o_sb = smp.tile([65, ICW], f32, tag="osb", name="osb")
                    nc.vector.tensor_copy(o_sb[:], o_ps[hh][0:65, :])
                    lrow = smp.tile([1, ICW], f32, tag="lrow", name="lrow")
                    nc.sync.dma_start(lrow[:], o_sb[64:65, :])
                    linv = smp.tile([1, ICW], f32, tag="linv", name="linv")
                    lsc = smp.tile([1, ICW], f32, tag="lsc", name="lsc")
                    nc.vector.reciprocal_approx_accurate(linv[:], lrow[:],
                                                         lsc[:])
                    lbc = smp.tile([64, ICW], f32, tag="lbc", name="lbc")
                    nc.gpsimd.partition_broadcast(lbc[:], linv[:])
                    if hh == 0:
                        nc.vector.tensor_mul(o_norm[pair][ic][0:64, :],
                                             o_sb[0:64, :], lbc[:])
                    else:
                        ob = smp.tile([64, ICW], bf16, tag="ob", name="ob")
                        nc.vector.tensor_mul(ob[:], o_sb[0:64, :], lbc[:])
                        nc.sync.dma_start(o_norm[pair][ic][64:128, :], ob[:])
                # output projection for this i-chunk (after both pairs done)
                if pair == 1:
                    for mt in range(MT):
                        msl = slice(mt * 128, (mt + 1) * 128)
                        pps = psum.tile([128, ICW], f32, tag="g", name="pj_ps")
                        for d in range(2):
                            nc.tensor.matmul(pps[:], wo[d][:, msl],
                                             o_norm[d][ic][:],
                                             start=(d == 0), stop=(d == 1))
                        ot = otp.tile([128, ICW], bf16, tag="ot", name="ot")
                        nc.vector.tensor_copy(ot[:], pps[:])
                        nc.sync.dma_start(out_d[msl, isl], ot[:])

    nc.compile()
    return nc


def _get_compiled(has_wb):
    key = bool(has_wb)
    if key not in _COMPILED:
        _COMPILED[key] = _build(key)
    return _COMPILED[key]


def _shard_inputs(x, ln_gamma, ln_beta, w_qkv, w_out):
    """Build per-core input maps (all host-side layout work, no math on x)."""
    import ml_dtypes
    bf = ml_dtypes.bfloat16

    x = np.ascontiguousarray(np.asarray(x, np.float32))
    g = np.asarray(ln_gamma, np.float32)
    be = np.asarray(ln_beta, np.float32)
    wq = np.asarray(w_qkv, np.float32)
    wo = np.asarray(w_out, np.float32)

    scale = DIM_HEAD ** (-0.5)
    wq_g = wq * g[:, None]            # gamma folded
    wq_g[:, :INNER] *= scale          # attention scale folded into W_q
    wb_full = be @ wq                 # beta contribution
    wb_full[:INNER] *= scale

    in_maps = []
    for c in range(N_CORES):
        b = c // HEADS_PER_CORE
        grp = c % HEADS_PER_CORE
        heads = [HEADS_PER_CORE * grp + t for t in range(HEADS_PER_CORE)]
        # column order: [q01, q23, k01, k23, v01, v23] pair-tiles
        cols = []
        for which in range(3):        # q, k, v
            for h in heads:
                lo = which * INNER + h * DIM_HEAD
                cols.append(np.arange(lo, lo + DIM_HEAD))
        cols = np.concatenate(cols)
        wqkv_c = np.ascontiguousarray(wq_g[:, cols])
        # bf16-round the weights before computing wsum so the LN-fold
        # correction matches what the device matmul actually sums.
        wqkv_bf = wqkv_c.astype(bf)
        wsum_c = wqkv_bf.astype(np.float32).sum(axis=0).reshape(NQKV, 1)
        wb_c = wb_full[cols].reshape(NQKV, 1)
        rows = np.concatenate([np.arange(h * DIM_HEAD, (h + 1) * DIM_HEAD)
                               for h in heads])
        wout_c = np.ascontiguousarray(wo[rows, :])
        in_maps.append({
            "xT": np.ascontiguousarray(x[b].T).astype(bf),
            "wqkv": wqkv_bf,
            "wout": wout_c.astype(bf),
            "wsum": np.ascontiguousarray(wsum_c),
            "wb": np.ascontiguousarray(wb_c),
        })
    return in_maps


def _run(inputs, trace=False):
    from concourse.bass_utils import run_bass_kernel_spmd

    in_maps = _shard_inputs(inputs["x"], inputs["ln_gamma"],
                            inputs["ln_beta"], inputs["w_qkv"],
                            inputs["w_out"])
    has_wb = bool(np.max(np.abs(in_maps[0]["wb"])) > 0)
    nc = _get_compiled(has_wb)
    res = run_bass_kernel_spmd(nc, in_maps, core_ids=list(range(N_CORES)),
                               trace=trace)
    b_out = np.asarray(inputs["b_out"], np.float32)
    outs = []
    for b in range(B):
        acc = np.zeros((DIM, N), np.float32)
        for grp in range(HEADS_PER_CORE):
            acc += res.results[b * HEADS_PER_CORE + grp]["outT"].astype(
                np.float32)
        outs.append(acc.T + b_out)
    out = np.stack(outs).astype(np.float32)
    return out, res


def kernel(**inputs):
    out, _ = _run(inputs, trace=False)
    return out
